# revision 8
# baseline (speedup 1.0000x reference)
"""GNN message-passing layer on 8 TRN2 NeuronCores.

Computes out = relu((adj^T @ x / deg) @ U^T) for N=8192 nodes, D=512 dims.

Sharding: columns of adj (= output rows) are split across the 8 cores;
x and U are replicated, so each core computes a [1024, 512] output slab
with no collectives.

Host-side restaging (layout shuffles + dtype packing): adj is 0/1, so it
is stored as fp8e4 (1 byte, exact) instead of int32 — 4x less HBM
traffic — and fed to the PE directly as the fp8 moving operand against
bf16 x weights (mixed non-fp32 matmul dtypes run at full rate). x and U
are pre-cast to bf16 on the host (same rounding the device cast would
do). Every DRAM tensor is partition-major so each SBUF partition reads
one long contiguous run.

Per-core kernel (accumulating in f32 PSUM), two passes over the 1024
output rows (PSUM holds 4 d-chunks of 512 columns):
  aggT[d, i] = sum_j x[j, d] * A[j, i]   via x-chunk weights, A streamed
  deg[i]     = sum_j A[j, i]             fp8 per-partition partials on the
                                         DVE, partition-summed by ones-weight
                                         matmuls, PE-transposed to
                                         per-partition layout
  out[i, k]  = relu((sum_d aggT[d, i] * U^T[d, k]) / deg[i])
               (1/deg rides the Relu activation's per-partition scale)

The first T8 of the 64 contraction tiles run as fp8 DoubleRow pairs
(x quantized to e4m3 for those j-rows only): 2 k-tiles per PE pass.
The fp8 quantization error on a quarter of the contraction keeps the
output rel-err ~1.4e-2, under the 2e-2 gate (measured; inputs are
deterministic).
"""

import sys

if "/opt/trn_rl_repo" not in sys.path:
    sys.path.insert(0, "/opt/trn_rl_repo")

import ml_dtypes
import numpy as np

import concourse.bacc as bacc
from concourse.bass import _add_dep_helper
import concourse.mybir as mybir
import concourse.tile as tile
from concourse.bass_utils import run_bass_kernel_spmd

N = 8192          # nodes
D = 512           # node dim
NCORES = 8
SH = N // NCORES  # 1024 adj columns (output rows) per core
NJ = N // 128     # 64 contraction tiles
XG = 8            # j-tiles per load group
NG = NJ // XG     # 8 groups
T8 = 0            # leading k-tiles computed as fp8 DoubleRow pairs
N8G = T8 // XG    # leading groups that are entirely DoubleRow
F32 = mybir.dt.float32
BF16 = mybir.dt.bfloat16
F8E4 = mybir.dt.float8e4

_compiled = None


def _build():
    nc = bacc.Bacc("TRN2", target_bir_lowering=False, debug=False, num_devices=NCORES)
    # partition-major layouts (see _run for the host-side shuffles)
    x_d = nc.dram_tensor("x", [128, NJ - T8, D], BF16, kind="ExternalInput").ap()
    adj_d = nc.dram_tensor("adj", [2, 128, NJ, D], F8E4, kind="ExternalInput").ap()
    ut_d = nc.dram_tensor("ut", [128, 4, D], BF16, kind="ExternalInput").ap()
    out_d = nc.dram_tensor("out", [128, 8, D], F32, kind="ExternalOutput").ap()
    if T8:
        x8_d = nc.dram_tensor("x8", [128, T8, D], F8E4, kind="ExternalInput").ap()

    with tile.TileContext(nc) as tc:
        with (
            tc.tile_pool(name="xw", bufs=1) as xw_pool,
            tc.tile_pool(name="abf", bufs=10) as abf_pool,
            tc.tile_pool(name="cons", bufs=1) as cons_pool,
            tc.tile_pool(name="evac", bufs=2) as evac_pool,
            tc.tile_pool(name="osb", bufs=2) as osb_pool,
            tc.tile_pool(name="pacc", bufs=1, space="PSUM") as pacc_pool,
            tc.tile_pool(name="pout", bufs=2, space="PSUM") as pout_pool,
        ):
            ones = cons_pool.tile([128, D], BF16)
            nc.vector.memset(ones[:], 1.0)
            # f32 identity for PE-transpose of the deg row
            ident = cons_pool.tile([128, 128], F32)
            nc.vector.memset(ident[:], 1.0)
            nc.gpsimd.affine_select(
                ident[:], ident[:], pattern=[[-1, 128]], base=0,
                channel_multiplier=1,
                compare_op=mybir.AluOpType.is_equal, fill=0.0,
            )
            u_bf = cons_pool.tile([128, 4, D], BF16)
            if T8:
                x8w = cons_pool.tile([128, T8, D], F8E4, name="x8w")

            # dummy matmuls: PE filler issued where the first groups would
            # otherwise idle waiting on DMA; also warms the HAM clock gate
            dummy_ps = pacc_pool.tile([128, D], F32, tag="deg", name="dummy")

            def pe_filler(n):
                for _ in range(n):
                    nc.tensor.matmul(
                        dummy_ps[:], ones[:, 0:128], ones[:],
                        start=True, stop=True, skip_group_check=True,
                    )

            xg_tiles = [None] * NG

            def load_x_group(g, split=1):
                # x rides the sync HWDGE ring (separate from the adj ring)
                xg = xw_pool.tile([128, XG, D], BF16, tag=f"xg{g}", name=f"xg{g}")
                step = XG // split
                for s in range(split):
                    nc.sync.dma_start(
                        xg[:, s * step:(s + 1) * step, :],
                        x_d[:, (g - N8G) * XG + s * step:
                               (g - N8G) * XG + (s + 1) * step, :],
                    )
                xg_tiles[g] = xg

            prev_recipt = None
            for h in range(2):
                agg_ps = [
                    pacc_pool.tile([128, D], F32, tag=f"agg{c}", name=f"agg{c}")
                    for c in range(4)
                ]
                agg_sc = [
                    evac_pool.tile([128, D], BF16, tag=f"aggsc{c}", name=f"aggsc{c}")
                    for c in range(4)
                ]
                # per-partition partial degree counts; values stay <= NJ so
                # bf16 accumulation is exact
                degp = evac_pool.tile([128, D], BF16, tag="degp", bufs=2)
                ms = nc.vector.memset(degp[:], 0.0)
                if prev_recipt is not None:
                    # keep the DVE FIFO from running this half's degp chain
                    # ahead of the previous half's recip (head-of-line block)
                    _add_dep_helper(ms.ins, prev_recipt.ins, sync=True,
                                    reason="degp chain after prev recip")
                if h == 0:
                    if T8:
                        nc.sync.dma_start(x8w[:], x8_d[:])
                    pe_filler(5)
                for g in range(NG):
                    if h == 0 and g >= N8G:
                        load_x_group(g, split=2 if g == N8G else 1)
                    a_bf = abf_pool.tile([128, XG, D], F8E4, tag="abf")
                    # adj rides the scalar HWDGE ring (qActDynamicHW) — the
                    # SWDGE path is ~2x slower. The very first tiles go on
                    # the sync ring, whose queue opens earliest, so the PE
                    # can start real matmuls sooner.
                    if h == 0 and g == 0:
                        nc.sync.dma_start(a_bf[:, 0:2, :], adj_d[0, :, 0:2, :])
                        nc.scalar.dma_start(a_bf[:, 2:XG, :], adj_d[0, :, 2:XG, :])
                    else:
                        nc.scalar.dma_start(
                            a_bf[:], adj_d[h, :, g * XG:(g + 1) * XG, :]
                        )
                    if h == 0 and g == 0:
                        nc.gpsimd.dma_start(u_bf[:], ut_d[:])
                    for ti in range(XG):
                        nc.vector.tensor_add(degp[:], degp[:], a_bf[:, ti, :])
                    if g < N8G:
                        # fp8 DoubleRow: two k-tiles per PE pass
                        for pt in range(XG // 2):
                            t = g * XG + 2 * pt
                            for c in range(4):
                                nc.tensor.matmul(
                                    agg_ps[c][:],
                                    x8w[:, t:t + 2, c * 128:(c + 1) * 128],
                                    a_bf[:, 2 * pt:2 * pt + 2, :],
                                    start=t == 0,
                                    stop=False,
                                    perf_mode=mybir.MatmulPerfMode.DoubleRow,
                                )
                    else:
                        xg = xg_tiles[g]
                        for ti in range(XG):
                            t = g * XG + ti
                            st, sp = t == 0, t == NJ - 1
                            for c in range(4):
                                nc.tensor.matmul(
                                    agg_ps[c][:],
                                    xg[:, ti, c * 128:(c + 1) * 128],
                                    a_bf[:, ti, :],
                                    start=st,
                                    stop=sp,
                                )
                                if sp:
                                    # evacuate each chunk as soon as its
                                    # accumulation closes (overlaps the
                                    # remaining chunks' matmuls); on ACT so
                                    # the DVE FIFO can never block stage 2
                                    nc.scalar.copy(agg_sc[c][:], agg_ps[c][:])

                # deg pipeline: partition-sum the partial counts with a
                # ones-weight matmul, then transpose into per-partition
                # layout for the output scale
                deg_ps = pacc_pool.tile([128, D], F32, tag="deg")
                nc.tensor.matmul(
                    deg_ps[:], ones[:, 0:128], degp[:],
                    start=True, stop=True,
                )
                deg_sb = evac_pool.tile([128, D], F32, tag="degsb")
                nc.scalar.copy(deg_sb[:], deg_ps[:])
                degt_ps = pacc_pool.tile([128, 4, 128], F32, tag="deg")
                for ic in range(4):
                    nc.tensor.transpose(
                        degt_ps[:, ic, :],
                        deg_sb[:, ic * 128:(ic + 1) * 128],
                        ident[:],
                    )
                recipt = evac_pool.tile([128, 4], F32, tag="recipt")
                prev_recipt = nc.vector.reciprocal_approx_fast(
                    recipt[:], degt_ps[:, :, 0]
                )

                out_sb = osb_pool.tile([128, 4, D], F32, tag="osb")
                for ic in range(4):
                    out_ps = pout_pool.tile([128, D], F32, tag="outps")
                    for c in range(4):
                        nc.tensor.matmul(
                            out_ps[:],
                            agg_sc[c][:, ic * 128:(ic + 1) * 128],
                            u_bf[:, c, :],
                            start=c == 0,
                            stop=c == 3,
                        )
                    # out = relu(out_raw / deg): positive scale commutes
                    # with relu, applied per partition in the activation
                    nc.scalar.activation(
                        out_sb[:, ic, :], out_ps[:],
                        mybir.ActivationFunctionType.Relu,
                        scale=recipt[:, ic:ic + 1],
                    )
                    nc.sync.dma_start(
                        out_d[:, h * 4 + ic, :], out_sb[:, ic, :]
                    )

    nc.compile()
    return nc


def _get_compiled():
    global _compiled
    if _compiled is None:
        _compiled = _build()
    return _compiled


def _run(x, adj, u, **spmd_kwargs):
    nc = _get_compiled()
    x = np.asarray(x, dtype=np.float32)
    adj = np.asarray(adj, dtype=np.int32)
    u = np.asarray(u, dtype=np.float32)

    # x[t*128+p, d] -> x_r[p, t, d], bf16 (same rounding the device cast did)
    x_r = np.ascontiguousarray(
        x.reshape(NJ, 128, D).transpose(1, 0, 2)
    ).astype(ml_dtypes.bfloat16)
    # U^T[c*128+p, k] -> ut_r[p, c, k]
    ut_r = np.ascontiguousarray(
        u.T.reshape(4, 128, D).transpose(1, 0, 2)
    ).astype(ml_dtypes.bfloat16)
    # adj is 0/1: pack to fp8e4 (1.0 == 0x38) — exact, 1 byte per entry
    adj8 = (adj.astype(np.uint8) * np.uint8(0x38)).view(ml_dtypes.float8_e4m3)
    in_common = {"x": np.ascontiguousarray(x_r[:, T8:, :]), "ut": ut_r}
    if T8:
        # leading T8 k-tiles use fp8 weights (quantized from f32, not bf16)
        x8_r = np.ascontiguousarray(
            x.reshape(NJ, 128, D).transpose(1, 0, 2)[:, :T8, :]
        ).astype(ml_dtypes.float8_e4m3)
        in_common["x8"] = x8_r
    in_maps = []
    for core in range(NCORES):
        shard = adj8[:, core * SH:(core + 1) * SH]
        # shard[t*128+p, h*512+d] -> adj_r[h, p, t, d]
        adj_r = np.ascontiguousarray(
            shard.reshape(NJ, 128, 2, D).transpose(2, 1, 0, 3)
        )
        in_maps.append({**in_common, "adj": adj_r})

    res = run_bass_kernel_spmd(nc, in_maps, core_ids=list(range(NCORES)), **spmd_kwargs)
    # out_r[p, hic, k] -> out[hic*128+p, k], then stack core slabs
    out = np.concatenate(
        [
            res.results[c]["out"].transpose(1, 0, 2).reshape(SH, D)
            for c in range(NCORES)
        ],
        axis=0,
    )
    return out, res


def kernel(x, adj, U):
    out, _ = _run(x, adj, U)
    return out


# revision 12
# speedup vs baseline: 1.0622x; 1.0622x over previous
"""GNN message-passing layer on 8 TRN2 NeuronCores.

Computes out = relu((adj^T @ x / deg) @ U^T) for N=8192 nodes, D=512 dims.

Sharding: columns of adj (= output rows) are split across the 8 cores;
x and U are replicated, so each core computes a [1024, 512] output slab
with no collectives.

Host-side restaging (layout shuffles + dtype packing): adj is 0/1, so it
is stored as fp8e4 (1 byte, exact) instead of int32 — 4x less HBM
traffic — and fed to the PE directly as the fp8 moving operand against
bf16 x weights (mixed non-fp32 matmul dtypes run at full rate). x and U
are pre-cast to bf16 on the host (same rounding the device cast would
do). Every DRAM tensor is partition-major so each SBUF partition reads
one long contiguous run.

Per-core kernel (accumulating in f32 PSUM), two passes over the 1024
output rows (PSUM holds 4 d-chunks of 512 columns):
  aggT[d, i] = sum_j x[j, d] * A[j, i]   via x-chunk weights, A streamed
  deg[i]     = sum_j A[j, i]             fp8 per-partition partials on the
                                         DVE, partition-summed by ones-weight
                                         matmuls, PE-transposed to
                                         per-partition layout
  out[i, k]  = relu((sum_d aggT[d, i] * U^T[d, k]) / deg[i])
               (1/deg rides the Relu activation's per-partition scale)

The first T8 of the 64 contraction tiles run as fp8 DoubleRow pairs
(x quantized to e4m3 for those j-rows only): 2 k-tiles per PE pass.
The fp8 quantization error on a quarter of the contraction keeps the
output rel-err ~1.4e-2, under the 2e-2 gate (measured; inputs are
deterministic).
"""

import sys

if "/opt/trn_rl_repo" not in sys.path:
    sys.path.insert(0, "/opt/trn_rl_repo")

import ml_dtypes
import numpy as np

import concourse.bacc as bacc
from concourse.bass import _add_dep_helper
import concourse.mybir as mybir
import concourse.tile as tile
from concourse.bass_utils import run_bass_kernel_spmd

N = 8192          # nodes
D = 512           # node dim
NCORES = 8
SH = N // NCORES  # 1024 adj columns (output rows) per core
NJ = N // 128     # 64 contraction tiles
XG = 8            # j-tiles per load group
NG = NJ // XG     # 8 groups
T8 = 0            # leading k-tiles computed as fp8 DoubleRow pairs
N8G = T8 // XG    # leading groups that are entirely DoubleRow
F32 = mybir.dt.float32
BF16 = mybir.dt.bfloat16
F8E4 = mybir.dt.float8e4

_compiled = None


def _build():
    nc = bacc.Bacc("TRN2", target_bir_lowering=False, debug=False, num_devices=NCORES)
    # partition-major layouts (see _run for the host-side shuffles)
    x_d = nc.dram_tensor("x", [128, NJ - T8, D], BF16, kind="ExternalInput").ap()
    adj_d = nc.dram_tensor("adj", [2, 128, NJ, D], F8E4, kind="ExternalInput").ap()
    ut_d = nc.dram_tensor("ut", [128, 4, D], BF16, kind="ExternalInput").ap()
    out_d = nc.dram_tensor("out", [128, 8, D], F32, kind="ExternalOutput").ap()
    if T8:
        x8_d = nc.dram_tensor("x8", [128, T8, D], F8E4, kind="ExternalInput").ap()

    with tile.TileContext(nc) as tc:
        with (
            tc.tile_pool(name="xw", bufs=1) as xw_pool,
            tc.tile_pool(name="abf", bufs=10) as abf_pool,
            tc.tile_pool(name="cons", bufs=1) as cons_pool,
            tc.tile_pool(name="evac", bufs=2) as evac_pool,
            tc.tile_pool(name="osb", bufs=2) as osb_pool,
            tc.tile_pool(name="pacc", bufs=1, space="PSUM") as pacc_pool,
            tc.tile_pool(name="pout", bufs=2, space="PSUM") as pout_pool,
        ):
            ones = cons_pool.tile([128, D], BF16)
            nc.vector.memset(ones[:], 1.0)
            # f32 identity for PE-transpose of the deg row
            ident = cons_pool.tile([128, 128], F32)
            nc.vector.memset(ident[:], 1.0)
            nc.gpsimd.affine_select(
                ident[:], ident[:], pattern=[[-1, 128]], base=0,
                channel_multiplier=1,
                compare_op=mybir.AluOpType.is_equal, fill=0.0,
            )
            u_bf = cons_pool.tile([128, 4, D], BF16)
            if T8:
                x8w = cons_pool.tile([128, T8, D], F8E4, name="x8w")

            # dummy matmuls: PE filler issued where the first groups would
            # otherwise idle waiting on DMA; also warms the HAM clock gate
            dummy_ps = pacc_pool.tile([128, D], F32, tag="deg", name="dummy")

            def pe_filler(n):
                for _ in range(n):
                    nc.tensor.matmul(
                        dummy_ps[:], ones[:, 0:128], ones[:],
                        start=True, stop=True, skip_group_check=True,
                    )

            # variable group sizes: tiny leading groups so the very first
            # DMAs are small and the PE can start matmuls early (deps are
            # tile-granular — a tile only becomes readable once its whole
            # DMA lands)
            GS = [2, 6] + [XG] * 7
            GS_H = [GS, GS]
            xg_tiles = {}

            def load_x_group(h, g, t0, gsz):
                # x rides the sync HWDGE ring (separate from the adj ring)
                xg = xw_pool.tile([128, gsz, D], BF16, tag=f"xg{g}", name=f"xg{g}")
                nc.sync.dma_start(xg[:], x_d[:, t0 - T8:t0 - T8 + gsz, :])
                xg_tiles[g] = xg

            prev_recipt = None
            for h in range(2):
                agg_ps = [
                    pacc_pool.tile([128, D], F32, tag=f"agg{c}", name=f"agg{c}")
                    for c in range(4)
                ]
                agg_sc = [
                    evac_pool.tile([128, D], BF16, tag=f"aggsc{c}", name=f"aggsc{c}")
                    for c in range(4)
                ]
                # per-partition partial degree counts; values stay <= NJ so
                # bf16 accumulation is exact
                degp = evac_pool.tile([128, D], BF16, tag="degp", bufs=2)
                ms = nc.vector.memset(degp[:], 0.0)
                if prev_recipt is not None:
                    # keep the DVE FIFO from running this half's degp chain
                    # ahead of the previous half's recip (head-of-line block)
                    _add_dep_helper(ms.ins, prev_recipt.ins, sync=True,
                                    reason="degp chain after prev recip")
                if h == 0:
                    if T8:
                        nc.sync.dma_start(x8w[:], x8_d[:])
                    pe_filler(3)
                t0 = 0
                for g, gsz in enumerate(GS_H[h]):
                    if h == 0 and t0 >= T8:
                        load_x_group(h, g, t0, gsz)
                    a_bf = abf_pool.tile(
                        [128, gsz, D], F8E4, tag=f"abf{gsz}",
                        bufs=10 if gsz == XG else 1,
                    )
                    # adj rides the scalar HWDGE ring (qActDynamicHW) — the
                    # SWDGE path is ~2x slower
                    nc.scalar.dma_start(
                        a_bf[:], adj_d[h, :, t0:t0 + gsz, :]
                    )
                    if h == 0 and g == 0:
                        nc.gpsimd.dma_start(u_bf[:], ut_d[:])
                    for ti in range(gsz):
                        nc.vector.tensor_add(degp[:], degp[:], a_bf[:, ti, :])
                    if t0 < T8:
                        # fp8 DoubleRow: two k-tiles per PE pass
                        for pt in range(gsz // 2):
                            t = t0 + 2 * pt
                            for c in range(4):
                                nc.tensor.matmul(
                                    agg_ps[c][:],
                                    x8w[:, t:t + 2, c * 128:(c + 1) * 128],
                                    a_bf[:, 2 * pt:2 * pt + 2, :],
                                    start=t == 0,
                                    stop=False,
                                    perf_mode=mybir.MatmulPerfMode.DoubleRow,
                                )
                    else:
                        xg = xg_tiles[g]
                        for ti in range(gsz):
                            t = t0 + ti
                            st, sp = t == 0, t == NJ - 1
                            for c in range(4):
                                nc.tensor.matmul(
                                    agg_ps[c][:],
                                    xg[:, ti, c * 128:(c + 1) * 128],
                                    a_bf[:, ti, :],
                                    start=st,
                                    stop=sp,
                                )
                                if sp:
                                    # evacuate each chunk as soon as its
                                    # accumulation closes (overlaps the
                                    # remaining chunks' matmuls); on ACT so
                                    # the DVE FIFO can never block stage 2
                                    nc.scalar.copy(agg_sc[c][:], agg_ps[c][:])
                    if h == 0 and g < 2:
                        pe_filler(2)
                    t0 += gsz

                # deg pipeline: partition-sum the partial counts with a
                # ones-weight matmul, then transpose into per-partition
                # layout for the output scale
                deg_ps = pacc_pool.tile([128, D], F32, tag="deg")
                nc.tensor.matmul(
                    deg_ps[:], ones[:, 0:128], degp[:],
                    start=True, stop=True,
                )
                deg_sb = evac_pool.tile([128, D], F32, tag="degsb")
                nc.scalar.copy(deg_sb[:], deg_ps[:])
                degt_ps = pacc_pool.tile([128, 4, 128], F32, tag="deg")
                for ic in range(4):
                    nc.tensor.transpose(
                        degt_ps[:, ic, :],
                        deg_sb[:, ic * 128:(ic + 1) * 128],
                        ident[:],
                    )
                recipt = evac_pool.tile([128, 4], F32, tag="recipt")
                prev_recipt = nc.vector.reciprocal_approx_fast(
                    recipt[:], degt_ps[:, :, 0]
                )

                out_sb = osb_pool.tile([128, 4, D], F32, tag="osb")
                for ic in range(4):
                    out_ps = pout_pool.tile([128, D], F32, tag="outps")
                    for c in range(4):
                        nc.tensor.matmul(
                            out_ps[:],
                            agg_sc[c][:, ic * 128:(ic + 1) * 128],
                            u_bf[:, c, :],
                            start=c == 0,
                            stop=c == 3,
                        )
                    # out = relu(out_raw / deg): positive scale commutes
                    # with relu, applied per partition in the activation
                    nc.scalar.activation(
                        out_sb[:, ic, :], out_ps[:],
                        mybir.ActivationFunctionType.Relu,
                        scale=recipt[:, ic:ic + 1],
                    )
                    nc.sync.dma_start(
                        out_d[:, h * 4 + ic, :], out_sb[:, ic, :]
                    )

    nc.compile()
    return nc


def _get_compiled():
    global _compiled
    if _compiled is None:
        _compiled = _build()
    return _compiled


def _run(x, adj, u, **spmd_kwargs):
    nc = _get_compiled()
    x = np.asarray(x, dtype=np.float32)
    adj = np.asarray(adj, dtype=np.int32)
    u = np.asarray(u, dtype=np.float32)

    # x[t*128+p, d] -> x_r[p, t, d], bf16 (same rounding the device cast did)
    x_r = np.ascontiguousarray(
        x.reshape(NJ, 128, D).transpose(1, 0, 2)
    ).astype(ml_dtypes.bfloat16)
    # U^T[c*128+p, k] -> ut_r[p, c, k]
    ut_r = np.ascontiguousarray(
        u.T.reshape(4, 128, D).transpose(1, 0, 2)
    ).astype(ml_dtypes.bfloat16)
    # adj is 0/1: pack to fp8e4 (1.0 == 0x38) — exact, 1 byte per entry
    adj8 = (adj.astype(np.uint8) * np.uint8(0x38)).view(ml_dtypes.float8_e4m3)
    in_common = {"x": np.ascontiguousarray(x_r[:, T8:, :]), "ut": ut_r}
    if T8:
        # leading T8 k-tiles use fp8 weights (quantized from f32, not bf16)
        x8_r = np.ascontiguousarray(
            x.reshape(NJ, 128, D).transpose(1, 0, 2)[:, :T8, :]
        ).astype(ml_dtypes.float8_e4m3)
        in_common["x8"] = x8_r
    in_maps = []
    for core in range(NCORES):
        shard = adj8[:, core * SH:(core + 1) * SH]
        # shard[t*128+p, h*512+d] -> adj_r[h, p, t, d]
        adj_r = np.ascontiguousarray(
            shard.reshape(NJ, 128, 2, D).transpose(2, 1, 0, 3)
        )
        in_maps.append({**in_common, "adj": adj_r})

    res = run_bass_kernel_spmd(nc, in_maps, core_ids=list(range(NCORES)), **spmd_kwargs)
    # out_r[p, hic, k] -> out[hic*128+p, k], then stack core slabs
    out = np.concatenate(
        [
            res.results[c]["out"].transpose(1, 0, 2).reshape(SH, D)
            for c in range(NCORES)
        ],
        axis=0,
    )
    return out, res


def kernel(x, adj, U):
    out, _ = _run(x, adj, U)
    return out
